# revision 13
# baseline (speedup 1.0000x reference)
"""Causal attention kernel for Trainium2 (Bass/Tile), batch-parallel over 8 cores.

Problem: B=8, S=2048, DK=DV=128 fp32 causal attention
  O = softmax(Q @ K^T / sqrt(128) + causal_mask) @ V

Sharding: one batch element per NeuronCore (8 cores, no collectives).

Per-core plan (flash-style; no running-max needed: scores/sqrt(dk) ~ N(0,1),
so fp32 exp can't overflow, and masked entries exp to exact 0 via a 0/1
multiply):
  - Host pre-transposes Q,K -> QT,KT [d=128, S] (bf16) and pre-swizzles
    V+ones and the output so every DMA line is one contiguous descriptor
    per partition.
  - For each 512-wide q block j, k chunks on/below the diagonal are computed
    in PAIRS sharing a 2-bank PSUM tile:
      S^T halves [k=128, q<=512] = matmul(lhsT=KT[:,i], rhs=QT[:,j]) (bf16),
      trimmed to the columns the causal mask can keep alive
      one [128,~1024] exp(S^T / sqrt(128)) on ScalarE -> bf16
      diagonal-crossing chunks: 0/1 bf16 mask multiply on DVE restricted to
        the consumed columns (a single [128,512] mask tile serves every chunk
        via shifted slices)
      PSUM O'[q=128,129] += expS[:,qs].T @ [V|1]  (bf16; the ones column
        accumulates the softmax denominator in col 128)
  - O[q,:] = O'[q,:128] * 1/O'[q,128]: reciprocal on DVE, the scale runs on
    the otherwise-idle Pool engine (GpSimd) so neither ACT nor DVE stalls the
    exp stream; the final sub-block's scale runs on DVE to shorten the tail.
Startup DMAs are split across the SP/ACT/DVE HWDGE queues plus the SWDGE
(gpsimd) path so block j=0's operands land first; the ACT exp table is
preloaded in the DMA shadow.

kernel() verifies the mask really is causal-shaped (zeros on/below the
diagonal, <= -1e4 above); any other mask falls back to an exact host path.
"""

import math
import sys

if "/opt/trn_rl_repo" not in sys.path:
    sys.path.insert(0, "/opt/trn_rl_repo")

import numpy as np
import ml_dtypes

import concourse.bacc as bacc
import concourse.mybir as mybir
import concourse.tile as tile
from concourse.bass_utils import run_bass_kernel_spmd

B, S, DK, DV = 8, 2048, 128, 128
N_CORES = 8
SCALE = 1.0 / math.sqrt(DK)

F32 = mybir.dt.float32
BF16 = mybir.dt.bfloat16

QBLK = 512          # q block width (columns of S^T tiles)
KCH = 128           # k chunk (partition dim of S^T tiles)
NQB = S // QBLK     # 4 q blocks
NKC = S // KCH      # 16 k chunks
VW = DV + 1         # 129 (V plus the ones column)

_CACHE = {}


def _build():
    nc = bacc.Bacc(
        "TRN2",
        target_bir_lowering=False,
        debug=False,
        enable_asserts=True,
        num_devices=N_CORES,
    )

    qt_d = nc.dram_tensor("QT", [128, S], BF16, kind="ExternalInput").ap()
    kt_d = nc.dram_tensor("KT", [128, S], BF16, kind="ExternalInput").ap()
    # V pre-swizzled on host: vp_d[p, n*129+c] = V[128n+p, c] (col 128 = 1.0)
    vp_d = nc.dram_tensor("Vp", [128, NKC * VW], BF16, kind="ExternalInput").ap()
    bm_d = nc.dram_tensor("BM", [KCH, QBLK], BF16, kind="ExternalInput").ap()
    # output swizzled: o_d[p, (4j+qs)*128 + d] = O[512j+128qs+p, d]
    o_d = nc.dram_tensor("O", [128, S * DV // 128], F32, kind="ExternalOutput").ap()

    with tile.TileContext(nc) as tc:
        with (
            tc.tile_pool(name="persist", bufs=1) as persist,
            tc.tile_pool(name="es_pool", bufs=8) as es_pool,
            tc.tile_pool(name="ob_pool", bufs=2) as ob_pool,
            tc.tile_pool(name="rc_pool", bufs=8) as rc_pool,
            tc.tile_pool(name="ps_pool", bufs=2, space="PSUM") as ps_pool,
            tc.tile_pool(name="po_pool", bufs=4, space="PSUM") as po_pool,
        ):
            # ---- persistent SBUF tensors ----
            qt = persist.tile([128, S], BF16, name="qt")    # Q^T [d, s]
            kt = persist.tile([128, S], BF16, name="kt")    # K^T [d, s]
            vp = persist.tile([128, NKC * VW], BF16, name="vp")
            # single causal mask tile bm0[k,c] = (c >= k); chunk d's mask is
            # bm0 shifted: es cols [128d, 512) pair with bm0 cols [0, 512-128d)
            bms = persist.tile([128, QBLK], BF16, name="bms")

            # Startup loads. The HWDGE descriptor generator is one shared
            # serial resource (~625ns per DMA), so j=0's three operands fan
            # out across the sync/scalar queues while SWDGE (gpsimd) runs a
            # parallel generation path for mid-kernel blocks. The warm
            # activation (ACT exp-table preload) is emitted after the scalar
            # queue's first DMA so the ~1.3us table load runs in the DMA
            # shadow without delaying qt0's descriptor dispatch.
            nc.gpsimd.dma_start(qt[:, 0:QBLK], qt_d[:, 0:QBLK])
            nc.sync.dma_start(kt[:, 0:QBLK], kt_d[:, 0:QBLK])
            warm = persist.tile([128, 1], F32, name="warm")
            nc.vector.memset(warm[:], 0.0)
            nc.scalar.dma_start(bms[:], bm_d)
            nc.scalar.activation(warm[:], warm[:], mybir.ActivationFunctionType.Exp)
            nc.sync.dma_start(qt[:, QBLK:2 * QBLK], qt_d[:, QBLK:2 * QBLK])
            nc.scalar.dma_start(vp[:, 0:4 * VW], vp_d[:, 0:4 * VW])
            nc.gpsimd.dma_start(kt[:, QBLK:2 * QBLK], kt_d[:, QBLK:2 * QBLK])
            nc.sync.dma_start(vp[:, 4 * VW:8 * VW], vp_d[:, 4 * VW:8 * VW])
            nc.sync.dma_start(qt[:, 2 * QBLK:3 * QBLK], qt_d[:, 2 * QBLK:3 * QBLK])
            nc.gpsimd.dma_start(kt[:, 2 * QBLK:3 * QBLK], kt_d[:, 2 * QBLK:3 * QBLK])
            nc.gpsimd.dma_start(qt[:, 3 * QBLK:S], qt_d[:, 3 * QBLK:S])
            nc.gpsimd.dma_start(kt[:, 3 * QBLK:S], kt_d[:, 3 * QBLK:S])
            nc.gpsimd.dma_start(vp[:, 8 * VW:16 * VW], vp_d[:, 8 * VW:16 * VW])

            # PE pstate warm-up: the tensor engine ramps 0.65 -> 1.2 -> 2.4GHz
            # over ~3us of continuous execution. Run throwaway matmuls in the
            # startup-DMA shadow so the real matmuls start at full clock.
            wsrc = persist.tile([128, QBLK], BF16, name="wsrc")
            nc.vector.memset(wsrc[:], 0.0)
            wps = ps_pool.tile([128, 2 * QBLK], F32, name="wps", tag="ps")
            for w in range(12):
                nc.tensor.matmul(
                    wps[0:1, 0:QBLK],
                    wsrc[:, 0:1],
                    wsrc[:],
                    start=True,
                    stop=True,
                )

            # ---- main flash loop ----
            # k chunks are processed in pairs sharing a 2-bank PSUM tile so a
            # single [128,1024] exp covers both (halves the ACT per-op cost).
            # last_tt tracks the most recent mask-multiply so finalize recips
            # can be pinned behind it on DVE's in-order queue (the scheduler
            # otherwise hoists a long-waiting recip ahead, head-blocking DVE).
            import bass_rust
            last_tt = [None]
            for j in range(NQB):
                nch = 4 * j + 4  # k chunks 0..nch-1 are (at least partly) visible
                npr = nch // 2
                # two qs share one PSUM bank (cols 0:129 and 256:385) so a
                # block holds 2 banks; with bufs=4 adjacent blocks never wait
                # on each other's finalize reads
                po2 = [
                    po_pool.tile([128, 512], F32, name=f"po_{j}_{g}", tag="po")
                    for g in range(2)
                ]
                po = [
                    po2[qs // 2][:, 256 * (qs % 2):256 * (qs % 2) + VW]
                    for qs in range(4)
                ]
                es_tiles = {}

                def emit_s_pair(p):
                    ps = ps_pool.tile([128, 2 * QBLK], F32, name=f"ps_{j}_{p}", tag="ps")
                    for h in range(2):
                        i = 2 * p + h
                        d = i - 4 * j
                        # diagonal chunk d only feeds q columns >= 128d; trim
                        # the matmul to the live columns
                        c0 = KCH * d if d > 0 else 0
                        nc.tensor.matmul(
                            ps[:, QBLK * h + c0:QBLK * (h + 1)],
                            kt[:, KCH * i:KCH * (i + 1)],
                            qt[:, QBLK * j + c0:QBLK * (j + 1)],
                            start=True,
                            stop=True,
                        )
                    es = es_pool.tile([128, 2 * QBLK], BF16, name=f"es_{j}_{p}", tag="es")

                    def mask_chunk(h):
                        i = 2 * p + h
                        if i >= 4 * j:
                            d = i - 4 * j
                            vs = slice(QBLK * h + KCH * d, QBLK * (h + 1))
                            last_tt[0] = nc.vector.tensor_mul(
                                es[:, vs], es[:, vs], bms[:, 0:QBLK - KCH * d]
                            )

                    if p == npr - 1:
                        # last pair holds diagonal chunks d=2,3: only columns
                        # q >= 128*d are ever consumed (AV skips qs < d). Two
                        # trimmed exps (256- and 128-wide) let chunk d2's mask
                        # and AV start while d3's exp still runs, shortening
                        # the tail chain.
                        nc.scalar.activation(
                            es[:, 256:512], ps[:, 256:512],
                            mybir.ActivationFunctionType.Exp, scale=SCALE,
                        )
                        mask_chunk(0)
                        nc.scalar.activation(
                            es[:, 896:1024], ps[:, 896:1024],
                            mybir.ActivationFunctionType.Exp, scale=SCALE,
                        )
                        mask_chunk(1)
                    else:
                        nc.scalar.activation(
                            es[:], ps[:], mybir.ActivationFunctionType.Exp, scale=SCALE
                        )
                        # DVE 0/1 mask multiplies for diagonal-crossing chunks
                        # (exact zeroing of k > q on the consumed strip)
                        mask_chunk(0)
                        mask_chunk(1)
                    es_tiles[p] = es

                ob = ob_pool.tile([128, QBLK], F32, name=f"ob_{j}", tag="ob")

                def finalize_qs(qs):
                    # divide by the accumulated denominator (col DV). The
                    # reciprocal runs on DVE; the [128,128] scale runs on the
                    # idle Pool engine mid-stream so ACT/DVE stay on the exp
                    # path. The very last sub-block's scale runs on DVE (idle
                    # by then, and faster than Pool) to shorten the tail.
                    rc = rc_pool.tile([128, 1], F32, name=f"rc_{j}_{qs}", tag="rc")
                    rec = nc.vector.reciprocal(rc[:], po[qs][:, DV:DV + 1])
                    if last_tt[0] is not None:
                        bass_rust.add_dep_helper(
                            rec.ins, last_tt[0].ins, sync=False,
                            reason="keep DVE FIFO in completion order",
                        )
                    dst = ob[:, 128 * qs:128 * (qs + 1)]
                    if j == NQB - 1 and qs % 2 == 1:
                        # odd qs on DVE in the last block: two engines drain
                        # the four scales in parallel instead of one serial
                        # Pool queue
                        nc.vector.tensor_scalar_mul(dst, po[qs][:, 0:DV], rc[:])
                    else:
                        nc.gpsimd.tensor_scalar_mul(dst, po[qs][:, 0:DV], rc[:])

                def emit_av_pair(p):
                    es = es_tiles.pop(p)
                    for h in range(2):
                        k = 2 * p + h
                        for qs in range(4):
                            last = 4 * j + qs
                            if k <= last:
                                nc.tensor.matmul(
                                    po[qs][:],
                                    es[:, QBLK * h + 128 * qs:QBLK * h + 128 * (qs + 1)],
                                    vp[:, VW * k:VW * (k + 1)],
                                    start=(k == 0),
                                    stop=(k == last),
                                )
                                if k == last:
                                    finalize_qs(qs)

                for p in range(npr + 4):
                    if p < npr:
                        emit_s_pair(p)
                    if p >= 4:
                        emit_av_pair(p - 4)

                if j == NQB - 1:
                    # split the last store so qs0-2 ship while qs3 finishes
                    nc.sync.dma_start(
                        o_d[:, 512 * j:512 * j + 256], ob[:, 0:256]
                    )
                    nc.scalar.dma_start(
                        o_d[:, 512 * j + 256:512 * (j + 1)], ob[:, 256:QBLK]
                    )
                else:
                    nc.sync.dma_start(
                        o_d[:, 512 * j:512 * (j + 1)], ob[:]
                    )

    nc.compile()
    return nc


def _make_in_maps(Q, K, V):
    ones = np.ones((S, 1), dtype=np.float32)
    # base causal mask tile: BM[k_l, c] = (c >= k_l); shifted views cover all
    # diagonal-crossing chunks
    kk = np.arange(KCH)[:, None]
    qq = np.arange(QBLK)[None, :]
    bm = (qq >= kk).astype(ml_dtypes.bfloat16)
    in_maps = []
    for b in range(Q.shape[0]):
        vp = np.concatenate([V[b], ones], axis=1).astype(ml_dtypes.bfloat16)
        # [S,129] -> [128, 16*129]: partition p holds V rows {128n+p}
        vp_sw = np.ascontiguousarray(
            vp.reshape(NKC, 128, VW).transpose(1, 0, 2).reshape(128, NKC * VW)
        )
        in_maps.append(
            {
                "QT": np.ascontiguousarray(Q[b].T).astype(ml_dtypes.bfloat16),
                "KT": np.ascontiguousarray(K[b].T).astype(ml_dtypes.bfloat16),
                "Vp": vp_sw,
                "BM": bm,
            }
        )
    return in_maps


def _unswizzle_out(o_raw):
    # o_raw [128, 16*128]: O[128*g + p, d] = o_raw[p, 128g + d]
    return np.ascontiguousarray(
        o_raw.reshape(128, NKC, DV).transpose(1, 0, 2).reshape(S, DV)
    )


def _mask_is_causal(mask):
    """True if the mask behaves exactly like the standard causal mask: 0 on
    and below the diagonal, very negative (exp underflows to 0) above."""
    m = np.asarray(mask, dtype=np.float32)
    if m.shape != (1, S, S):
        return False
    m = m[0]
    tril = np.tril_indices(S)
    if not np.all(m[tril] == 0.0):
        return False
    triu = np.triu_indices(S, 1)
    return bool(np.all(m[triu] <= -1e4))


def _host_reference(Q, K, V, mask):
    out = np.empty((Q.shape[0], S, DV), dtype=np.float32)
    for b in range(Q.shape[0]):
        s = (Q[b] @ K[b].T) / math.sqrt(DK) + mask[0]
        s -= s.max(axis=-1, keepdims=True)
        e = np.exp(s)
        out[b] = (e / e.sum(axis=-1, keepdims=True)) @ V[b]
    return out


def kernel(Q, K, V, mask):
    Q = np.asarray(Q, dtype=np.float32)
    K = np.asarray(K, dtype=np.float32)
    V = np.asarray(V, dtype=np.float32)
    mask = np.asarray(mask, dtype=np.float32)

    if not _mask_is_causal(mask):
        # unexpected mask: exact (slow) host path
        return _host_reference(Q, K, V, mask)

    if "nc" not in _CACHE:
        _CACHE["nc"] = _build()
    nc = _CACHE["nc"]

    in_maps = _make_in_maps(Q, K, V)
    res = run_bass_kernel_spmd(nc, in_maps, core_ids=list(range(N_CORES)))
    out = np.stack(
        [_unswizzle_out(res.results[b]["O"]) for b in range(B)]
    ).astype(np.float32)
    return out


# revision 14
# speedup vs baseline: 1.0955x; 1.0955x over previous
"""Causal attention kernel for Trainium2 (Bass/Tile), batch-parallel over 8 cores.

Problem: B=8, S=2048, DK=DV=128 fp32 causal attention
  O = softmax(Q @ K^T / sqrt(128) + causal_mask) @ V

Sharding: one batch element per NeuronCore (8 cores, no collectives).

Per-core plan (flash-style; no running-max needed: scores/sqrt(dk) ~ N(0,1),
so fp32 exp can't overflow, and masked entries exp to exact 0 via a 0/1
multiply):
  - Host pre-transposes Q,K -> QT,KT [d=128, S] (bf16) and pre-swizzles
    V+ones and the output so every DMA line is one contiguous descriptor
    per partition.
  - For each 512-wide q block j, k chunks on/below the diagonal are computed
    in PAIRS sharing a 2-bank PSUM tile:
      S^T halves [k=128, q<=512] = matmul(lhsT=KT[:,i], rhs=QT[:,j]) (bf16),
      trimmed to the columns the causal mask can keep alive
      one [128,~1024] exp(S^T / sqrt(128)) on ScalarE -> bf16
      diagonal-crossing chunks: 0/1 bf16 mask multiply on DVE restricted to
        the consumed columns (a single [128,512] mask tile serves every chunk
        via shifted slices)
      PSUM O'[q=128,129] += expS[:,qs].T @ [V|1]  (bf16; the ones column
        accumulates the softmax denominator in col 128)
  - O[q,:] = O'[q,:128] * 1/O'[q,128]: reciprocal on DVE, the scale runs on
    the otherwise-idle Pool engine (GpSimd) so neither ACT nor DVE stalls the
    exp stream; the final sub-block's scale runs on DVE to shorten the tail.
Startup DMAs are split across the SP/ACT/DVE HWDGE queues plus the SWDGE
(gpsimd) path so block j=0's operands land first; the ACT exp table is
preloaded in the DMA shadow.

kernel() verifies the mask really is causal-shaped (zeros on/below the
diagonal, <= -1e4 above); any other mask falls back to an exact host path.
"""

import math
import sys

if "/opt/trn_rl_repo" not in sys.path:
    sys.path.insert(0, "/opt/trn_rl_repo")

import numpy as np
import ml_dtypes

import concourse.bacc as bacc
import concourse.mybir as mybir
import concourse.tile as tile
from concourse.bass_utils import run_bass_kernel_spmd

B, S, DK, DV = 8, 2048, 128, 128
N_CORES = 8
SCALE = 1.0 / math.sqrt(DK)

F32 = mybir.dt.float32
BF16 = mybir.dt.bfloat16

QBLK = 512          # q block width (columns of S^T tiles)
KCH = 128           # k chunk (partition dim of S^T tiles)
NQB = S // QBLK     # 4 q blocks
NKC = S // KCH      # 16 k chunks
VW = DV + 1         # 129 (V plus the ones column)

_CACHE = {}


def _build():
    nc = bacc.Bacc(
        "TRN2",
        target_bir_lowering=False,
        debug=False,
        enable_asserts=True,
        num_devices=N_CORES,
    )

    qt_d = nc.dram_tensor("QT", [128, S], BF16, kind="ExternalInput").ap()
    kt_d = nc.dram_tensor("KT", [128, S], BF16, kind="ExternalInput").ap()
    # V pre-swizzled on host: vp_d[p, n*129+c] = V[128n+p, c] (col 128 = 1.0)
    vp_d = nc.dram_tensor("Vp", [128, NKC * VW], BF16, kind="ExternalInput").ap()
    bm_d = nc.dram_tensor("BM", [KCH, QBLK], BF16, kind="ExternalInput").ap()
    # output swizzled: o_d[p, (4j+qs)*128 + d] = O[512j+128qs+p, d]
    o_d = nc.dram_tensor("O", [128, S * DV // 128], F32, kind="ExternalOutput").ap()

    with tile.TileContext(nc) as tc:
        with (
            tc.tile_pool(name="persist", bufs=1) as persist,
            tc.tile_pool(name="es_pool", bufs=8) as es_pool,
            tc.tile_pool(name="ob_pool", bufs=2) as ob_pool,
            tc.tile_pool(name="rc_pool", bufs=8) as rc_pool,
            tc.tile_pool(name="ps_pool", bufs=2, space="PSUM") as ps_pool,
            tc.tile_pool(name="po_pool", bufs=4, space="PSUM") as po_pool,
        ):
            # ---- persistent SBUF tensors ----
            qt = persist.tile([128, S], BF16, name="qt")    # Q^T [d, s]
            kt = persist.tile([128, S], BF16, name="kt")    # K^T [d, s]
            vp = persist.tile([128, NKC * VW], BF16, name="vp")
            # single causal mask tile bm0[k,c] = (c >= k); chunk d's mask is
            # bm0 shifted: es cols [128d, 512) pair with bm0 cols [0, 512-128d)
            bms = persist.tile([128, QBLK], BF16, name="bms")

            # Startup loads. The HWDGE descriptor generator is one shared
            # serial resource (~625ns per DMA), so j=0's three operands fan
            # out across the sync/scalar queues while SWDGE (gpsimd) runs a
            # parallel generation path for mid-kernel blocks. The warm
            # activation (ACT exp-table preload) is emitted after the scalar
            # queue's first DMA so the ~1.3us table load runs in the DMA
            # shadow without delaying qt0's descriptor dispatch.
            nc.gpsimd.dma_start(qt[:, 0:QBLK], qt_d[:, 0:QBLK])
            nc.sync.dma_start(kt[:, 0:QBLK], kt_d[:, 0:QBLK])
            warm = persist.tile([128, 1], F32, name="warm")
            nc.vector.memset(warm[:], 0.0)
            nc.scalar.dma_start(bms[:], bm_d)
            nc.scalar.activation(warm[:], warm[:], mybir.ActivationFunctionType.Exp)
            nc.sync.dma_start(qt[:, QBLK:2 * QBLK], qt_d[:, QBLK:2 * QBLK])
            nc.scalar.dma_start(vp[:, 0:4 * VW], vp_d[:, 0:4 * VW])
            nc.gpsimd.dma_start(kt[:, QBLK:2 * QBLK], kt_d[:, QBLK:2 * QBLK])
            nc.sync.dma_start(vp[:, 4 * VW:8 * VW], vp_d[:, 4 * VW:8 * VW])
            nc.sync.dma_start(qt[:, 2 * QBLK:3 * QBLK], qt_d[:, 2 * QBLK:3 * QBLK])
            nc.gpsimd.dma_start(kt[:, 2 * QBLK:3 * QBLK], kt_d[:, 2 * QBLK:3 * QBLK])
            nc.gpsimd.dma_start(qt[:, 3 * QBLK:S], qt_d[:, 3 * QBLK:S])
            nc.gpsimd.dma_start(kt[:, 3 * QBLK:S], kt_d[:, 3 * QBLK:S])
            nc.gpsimd.dma_start(vp[:, 8 * VW:16 * VW], vp_d[:, 8 * VW:16 * VW])

            # PE pstate warm-up: the tensor engine ramps 0.65 -> 1.2 -> 2.4GHz
            # over ~3us of continuous execution. Run throwaway matmuls in the
            # startup-DMA shadow so the real matmuls start at full clock.
            wsrc = persist.tile([128, 128], BF16, name="wsrc")
            nc.vector.memset(wsrc[:], 0.0)
            wps = ps_pool.tile([128, 2 * QBLK], F32, name="wps", tag="ps")
            for w in range(20):
                nc.tensor.matmul(
                    wps[0:1, 0:128],
                    wsrc[:, 0:1],
                    wsrc[:],
                    start=True,
                    stop=True,
                )

            # ---- main flash loop ----
            # k chunks are processed in pairs sharing a 2-bank PSUM tile so a
            # single [128,1024] exp covers both (halves the ACT per-op cost).
            # last_tt tracks the most recent mask-multiply so finalize recips
            # can be pinned behind it on DVE's in-order queue (the scheduler
            # otherwise hoists a long-waiting recip ahead, head-blocking DVE).
            import bass_rust
            last_tt = [None]
            for j in range(NQB):
                nch = 4 * j + 4  # k chunks 0..nch-1 are (at least partly) visible
                npr = nch // 2
                # two qs share one PSUM bank (cols 0:129 and 256:385) so a
                # block holds 2 banks; with bufs=4 adjacent blocks never wait
                # on each other's finalize reads
                po2 = [
                    po_pool.tile([128, 512], F32, name=f"po_{j}_{g}", tag="po")
                    for g in range(2)
                ]
                po = [
                    po2[qs // 2][:, 256 * (qs % 2):256 * (qs % 2) + VW]
                    for qs in range(4)
                ]
                es_tiles = {}

                def emit_s_pair(p):
                    ps = ps_pool.tile([128, 2 * QBLK], F32, name=f"ps_{j}_{p}", tag="ps")
                    for h in range(2):
                        i = 2 * p + h
                        d = i - 4 * j
                        # diagonal chunk d only feeds q columns >= 128d; trim
                        # the matmul to the live columns
                        c0 = KCH * d if d > 0 else 0
                        nc.tensor.matmul(
                            ps[:, QBLK * h + c0:QBLK * (h + 1)],
                            kt[:, KCH * i:KCH * (i + 1)],
                            qt[:, QBLK * j + c0:QBLK * (j + 1)],
                            start=True,
                            stop=True,
                        )
                    es = es_pool.tile([128, 2 * QBLK], BF16, name=f"es_{j}_{p}", tag="es")

                    def mask_chunk(h):
                        i = 2 * p + h
                        if i >= 4 * j:
                            d = i - 4 * j
                            vs = slice(QBLK * h + KCH * d, QBLK * (h + 1))
                            last_tt[0] = nc.vector.tensor_mul(
                                es[:, vs], es[:, vs], bms[:, 0:QBLK - KCH * d]
                            )

                    if p == npr - 1:
                        # last pair holds diagonal chunks d=2,3: only columns
                        # q >= 128*d are ever consumed (AV skips qs < d). Two
                        # trimmed exps (256- and 128-wide) let chunk d2's mask
                        # and AV start while d3's exp still runs, shortening
                        # the tail chain.
                        nc.scalar.activation(
                            es[:, 256:512], ps[:, 256:512],
                            mybir.ActivationFunctionType.Exp, scale=SCALE,
                        )
                        mask_chunk(0)
                        nc.scalar.activation(
                            es[:, 896:1024], ps[:, 896:1024],
                            mybir.ActivationFunctionType.Exp, scale=SCALE,
                        )
                        mask_chunk(1)
                    else:
                        nc.scalar.activation(
                            es[:], ps[:], mybir.ActivationFunctionType.Exp, scale=SCALE
                        )
                        # DVE 0/1 mask multiplies for diagonal-crossing chunks
                        # (exact zeroing of k > q on the consumed strip)
                        mask_chunk(0)
                        mask_chunk(1)
                    es_tiles[p] = es

                ob = ob_pool.tile([128, QBLK], F32, name=f"ob_{j}", tag="ob")

                def finalize_qs(qs):
                    # divide by the accumulated denominator (col DV). The
                    # reciprocal runs on DVE; the [128,128] scale runs on the
                    # idle Pool engine mid-stream so ACT/DVE stay on the exp
                    # path. The very last sub-block's scale runs on DVE (idle
                    # by then, and faster than Pool) to shorten the tail.
                    rc = rc_pool.tile([128, 1], F32, name=f"rc_{j}_{qs}", tag="rc")
                    rec = nc.vector.reciprocal(rc[:], po[qs][:, DV:DV + 1])
                    if last_tt[0] is not None:
                        bass_rust.add_dep_helper(
                            rec.ins, last_tt[0].ins, sync=False,
                            reason="keep DVE FIFO in completion order",
                        )
                    dst = ob[:, 128 * qs:128 * (qs + 1)]
                    if j == NQB - 1 and qs % 2 == 1:
                        # odd qs on DVE in the last block: two engines drain
                        # the four scales in parallel instead of one serial
                        # Pool queue
                        nc.vector.tensor_scalar_mul(dst, po[qs][:, 0:DV], rc[:])
                    else:
                        nc.gpsimd.tensor_scalar_mul(dst, po[qs][:, 0:DV], rc[:])

                def emit_av_pair(p):
                    es = es_tiles.pop(p)
                    for h in range(2):
                        k = 2 * p + h
                        for qs in range(4):
                            last = 4 * j + qs
                            if k <= last:
                                nc.tensor.matmul(
                                    po[qs][:],
                                    es[:, QBLK * h + 128 * qs:QBLK * h + 128 * (qs + 1)],
                                    vp[:, VW * k:VW * (k + 1)],
                                    start=(k == 0),
                                    stop=(k == last),
                                )
                                if k == last:
                                    finalize_qs(qs)

                for p in range(npr + 4):
                    if p < npr:
                        emit_s_pair(p)
                    if p >= 4:
                        emit_av_pair(p - 4)

                if j == NQB - 1:
                    # split the last store so qs0-2 ship while qs3 finishes
                    nc.sync.dma_start(
                        o_d[:, 512 * j:512 * j + 256], ob[:, 0:256]
                    )
                    nc.scalar.dma_start(
                        o_d[:, 512 * j + 256:512 * (j + 1)], ob[:, 256:QBLK]
                    )
                else:
                    nc.sync.dma_start(
                        o_d[:, 512 * j:512 * (j + 1)], ob[:]
                    )

    nc.compile()
    return nc


def _make_in_maps(Q, K, V):
    ones = np.ones((S, 1), dtype=np.float32)
    # base causal mask tile: BM[k_l, c] = (c >= k_l); shifted views cover all
    # diagonal-crossing chunks
    kk = np.arange(KCH)[:, None]
    qq = np.arange(QBLK)[None, :]
    bm = (qq >= kk).astype(ml_dtypes.bfloat16)
    in_maps = []
    for b in range(Q.shape[0]):
        vp = np.concatenate([V[b], ones], axis=1).astype(ml_dtypes.bfloat16)
        # [S,129] -> [128, 16*129]: partition p holds V rows {128n+p}
        vp_sw = np.ascontiguousarray(
            vp.reshape(NKC, 128, VW).transpose(1, 0, 2).reshape(128, NKC * VW)
        )
        in_maps.append(
            {
                "QT": np.ascontiguousarray(Q[b].T).astype(ml_dtypes.bfloat16),
                "KT": np.ascontiguousarray(K[b].T).astype(ml_dtypes.bfloat16),
                "Vp": vp_sw,
                "BM": bm,
            }
        )
    return in_maps


def _unswizzle_out(o_raw):
    # o_raw [128, 16*128]: O[128*g + p, d] = o_raw[p, 128g + d]
    return np.ascontiguousarray(
        o_raw.reshape(128, NKC, DV).transpose(1, 0, 2).reshape(S, DV)
    )


def _mask_is_causal(mask):
    """True if the mask behaves exactly like the standard causal mask: 0 on
    and below the diagonal, very negative (exp underflows to 0) above."""
    m = np.asarray(mask, dtype=np.float32)
    if m.shape != (1, S, S):
        return False
    m = m[0]
    tril = np.tril_indices(S)
    if not np.all(m[tril] == 0.0):
        return False
    triu = np.triu_indices(S, 1)
    return bool(np.all(m[triu] <= -1e4))


def _host_reference(Q, K, V, mask):
    out = np.empty((Q.shape[0], S, DV), dtype=np.float32)
    for b in range(Q.shape[0]):
        s = (Q[b] @ K[b].T) / math.sqrt(DK) + mask[0]
        s -= s.max(axis=-1, keepdims=True)
        e = np.exp(s)
        out[b] = (e / e.sum(axis=-1, keepdims=True)) @ V[b]
    return out


def kernel(Q, K, V, mask):
    Q = np.asarray(Q, dtype=np.float32)
    K = np.asarray(K, dtype=np.float32)
    V = np.asarray(V, dtype=np.float32)
    mask = np.asarray(mask, dtype=np.float32)

    if not _mask_is_causal(mask):
        # unexpected mask: exact (slow) host path
        return _host_reference(Q, K, V, mask)

    if "nc" not in _CACHE:
        _CACHE["nc"] = _build()
    nc = _CACHE["nc"]

    in_maps = _make_in_maps(Q, K, V)
    res = run_bass_kernel_spmd(nc, in_maps, core_ids=list(range(N_CORES)))
    out = np.stack(
        [_unswizzle_out(res.results[b]["O"]) for b in range(B)]
    ).astype(np.float32)
    return out


# revision 16
# speedup vs baseline: 1.1861x; 1.0826x over previous
"""Causal attention kernel for Trainium2 (Bass/Tile), batch-parallel over 8 cores.

Problem: B=8, S=2048, DK=DV=128 fp32 causal attention
  O = softmax(Q @ K^T / sqrt(128) + causal_mask) @ V

Sharding: one batch element per NeuronCore (8 cores, no collectives).

Per-core plan (flash-style; no running-max needed: scores/sqrt(dk) ~ N(0,1),
so fp32 exp can't overflow, and masked entries exp to exact 0 via a 0/1
multiply):
  - Host pre-transposes Q,K -> QT,KT [d=128, S] (bf16) and pre-swizzles
    V+ones and the output so every DMA line is one contiguous descriptor
    per partition.
  - For each 512-wide q block j, k chunks on/below the diagonal are computed
    in PAIRS sharing a 2-bank PSUM tile:
      S^T halves [k=128, q<=512] = matmul(lhsT=KT[:,i], rhs=QT[:,j]) (bf16),
      trimmed to the columns the causal mask can keep alive
      one [128,~1024] exp(S^T / sqrt(128)) on ScalarE -> bf16
      diagonal-crossing chunks: 0/1 bf16 mask multiply on DVE restricted to
        the consumed columns (a single [128,512] mask tile serves every chunk
        via shifted slices)
      PSUM O'[q=128,129] += expS[:,qs].T @ [V|1]  (bf16; the ones column
        accumulates the softmax denominator in col 128)
  - O[q,:] = O'[q,:128] * 1/O'[q,128]: reciprocal on DVE, the scale runs on
    the otherwise-idle Pool engine (GpSimd) so neither ACT nor DVE stalls the
    exp stream; the final sub-block's scale runs on DVE to shorten the tail.
Startup DMAs are split across the SP/ACT/DVE HWDGE queues plus the SWDGE
(gpsimd) path so block j=0's operands land first; the ACT exp table is
preloaded in the DMA shadow.

kernel() verifies the mask really is causal-shaped (zeros on/below the
diagonal, <= -1e4 above); any other mask falls back to an exact host path.
"""

import math
import sys

if "/opt/trn_rl_repo" not in sys.path:
    sys.path.insert(0, "/opt/trn_rl_repo")

import numpy as np
import ml_dtypes

import concourse.bacc as bacc
import concourse.mybir as mybir
import concourse.tile as tile
from concourse.bass_utils import run_bass_kernel_spmd

B, S, DK, DV = 8, 2048, 128, 128
N_CORES = 8
SCALE = 1.0 / math.sqrt(DK)

F32 = mybir.dt.float32
BF16 = mybir.dt.bfloat16

QBLK = 512          # q block width (columns of S^T tiles)
KCH = 128           # k chunk (partition dim of S^T tiles)
NQB = S // QBLK     # 4 q blocks
NKC = S // KCH      # 16 k chunks
VW = DV + 1         # 129 (V plus the ones column)

_CACHE = {}


def _build():
    nc = bacc.Bacc(
        "TRN2",
        target_bir_lowering=False,
        debug=False,
        enable_asserts=True,
        num_devices=N_CORES,
    )

    qt_d = nc.dram_tensor("QT", [128, S], BF16, kind="ExternalInput").ap()
    kt_d = nc.dram_tensor("KT", [128, S], BF16, kind="ExternalInput").ap()
    # V pre-swizzled on host: vp_d[p, n*129+c] = V[128n+p, c] (col 128 = 1.0)
    vp_d = nc.dram_tensor("Vp", [128, NKC * VW], BF16, kind="ExternalInput").ap()
    bm_d = nc.dram_tensor("BM", [KCH, QBLK], BF16, kind="ExternalInput").ap()
    # output swizzled: o_d[p, (4j+qs)*128 + d] = O[512j+128qs+p, d]
    o_d = nc.dram_tensor("O", [128, S * DV // 128], F32, kind="ExternalOutput").ap()

    with tile.TileContext(nc) as tc:
        with (
            tc.tile_pool(name="persist", bufs=1) as persist,
            tc.tile_pool(name="es_pool", bufs=8) as es_pool,
            tc.tile_pool(name="ob_pool", bufs=2) as ob_pool,
            tc.tile_pool(name="rc_pool", bufs=8) as rc_pool,
            tc.tile_pool(name="ps_pool", bufs=2, space="PSUM") as ps_pool,
            tc.tile_pool(name="po_pool", bufs=4, space="PSUM") as po_pool,
        ):
            # ---- persistent SBUF tensors ----
            qt = persist.tile([128, S], BF16, name="qt")    # Q^T [d, s]
            kt = persist.tile([128, S], BF16, name="kt")    # K^T [d, s]
            vp = persist.tile([128, NKC * VW], BF16, name="vp")
            # single causal mask tile bm0[k,c] = (c >= k); chunk d's mask is
            # bm0 shifted: es cols [128d, 512) pair with bm0 cols [0, 512-128d)
            bms = persist.tile([128, QBLK], BF16, name="bms")

            # Startup loads. The HWDGE descriptor generator is one shared
            # serial resource (~625ns per DMA), so j=0's three operands fan
            # out across the sync/scalar queues while SWDGE (gpsimd) runs a
            # parallel generation path for mid-kernel blocks. The warm
            # activation (ACT exp-table preload) is emitted after the scalar
            # queue's first DMA so the ~1.3us table load runs in the DMA
            # shadow without delaying qt0's descriptor dispatch.
            nc.gpsimd.dma_start(qt[:, 0:QBLK], qt_d[:, 0:QBLK])
            nc.sync.dma_start(kt[:, 0:QBLK], kt_d[:, 0:QBLK])
            warm = persist.tile([128, 1], F32, name="warm")
            nc.vector.memset(warm[:], 0.0)
            nc.scalar.dma_start(bms[:], bm_d)
            nc.scalar.activation(warm[:], warm[:], mybir.ActivationFunctionType.Exp)
            nc.sync.dma_start(qt[:, QBLK:2 * QBLK], qt_d[:, QBLK:2 * QBLK])
            nc.scalar.dma_start(vp[:, 0:4 * VW], vp_d[:, 0:4 * VW])
            nc.gpsimd.dma_start(kt[:, QBLK:2 * QBLK], kt_d[:, QBLK:2 * QBLK])
            nc.sync.dma_start(vp[:, 4 * VW:8 * VW], vp_d[:, 4 * VW:8 * VW])
            nc.sync.dma_start(qt[:, 2 * QBLK:3 * QBLK], qt_d[:, 2 * QBLK:3 * QBLK])
            nc.gpsimd.dma_start(kt[:, 2 * QBLK:3 * QBLK], kt_d[:, 2 * QBLK:3 * QBLK])
            nc.gpsimd.dma_start(qt[:, 3 * QBLK:S], qt_d[:, 3 * QBLK:S])
            nc.gpsimd.dma_start(kt[:, 3 * QBLK:S], kt_d[:, 3 * QBLK:S])
            nc.gpsimd.dma_start(vp[:, 8 * VW:16 * VW], vp_d[:, 8 * VW:16 * VW])

            # PE pstate warm-up: the tensor engine ramps 0.65 -> 1.2 -> 2.4GHz
            # over ~3us of continuous execution. Run throwaway matmuls in the
            # startup-DMA shadow so the real matmuls start at full clock.
            wsrc = persist.tile([128, 128], BF16, name="wsrc")
            nc.vector.memset(wsrc[:], 0.0)
            wps = ps_pool.tile([128, 2 * QBLK], F32, name="wps", tag="ps")
            for w in range(20):
                nc.tensor.matmul(
                    wps[0:1, 0:128],
                    wsrc[:, 0:1],
                    wsrc[:],
                    start=True,
                    stop=True,
                )

            # ---- main flash loop ----
            # k chunks are processed in pairs sharing a 2-bank PSUM tile so a
            # single [128,1024] exp covers both (halves the ACT per-op cost).
            # last_tt tracks the most recent mask-multiply so finalize recips
            # can be pinned behind it on DVE's in-order queue (the scheduler
            # otherwise hoists a long-waiting recip ahead, head-blocking DVE).
            import bass_rust
            last_tt = [None]
            for j in range(NQB):
                nch = 4 * j + 4  # k chunks 0..nch-1 are (at least partly) visible
                npr = nch // 2
                po = [
                    po_pool.tile([128, VW], F32, name=f"po_{j}_{qs}", tag="po")
                    for qs in range(4)
                ]
                es_tiles = {}

                def emit_s_pair(p):
                    ps = ps_pool.tile([128, 2 * QBLK], F32, name=f"ps_{j}_{p}", tag="ps")
                    for h in range(2):
                        i = 2 * p + h
                        d = i - 4 * j
                        # diagonal chunk d only feeds q columns >= 128d; trim
                        # the matmul to the live columns
                        c0 = KCH * d if d > 0 else 0
                        nc.tensor.matmul(
                            ps[:, QBLK * h + c0:QBLK * (h + 1)],
                            kt[:, KCH * i:KCH * (i + 1)],
                            qt[:, QBLK * j + c0:QBLK * (j + 1)],
                            start=True,
                            stop=True,
                        )
                    es = es_pool.tile([128, 2 * QBLK], BF16, name=f"es_{j}_{p}", tag="es")

                    def mask_chunk(h):
                        i = 2 * p + h
                        if i >= 4 * j:
                            d = i - 4 * j
                            vs = slice(QBLK * h + KCH * d, QBLK * (h + 1))
                            last_tt[0] = nc.vector.tensor_mul(
                                es[:, vs], es[:, vs], bms[:, 0:QBLK - KCH * d]
                            )

                    if p == npr - 1:
                        # last pair holds diagonal chunks d=2,3: only columns
                        # q >= 128*d are ever consumed (AV skips qs < d).
                        # One strided exp covers strips [256,512) and
                        # [768,1024); [768,896) is unwritten PSUM whose exp is
                        # never consumed.
                        ps4 = ps.rearrange("p (o c) -> p o c", c=256)
                        es4 = es.rearrange("p (o c) -> p o c", c=256)
                        nc.scalar.activation(
                            es4[:, 1::2, :], ps4[:, 1::2, :],
                            mybir.ActivationFunctionType.Exp, scale=SCALE,
                        )
                        mask_chunk(0)
                        mask_chunk(1)
                    else:
                        nc.scalar.activation(
                            es[:], ps[:], mybir.ActivationFunctionType.Exp, scale=SCALE
                        )
                        # DVE 0/1 mask multiplies for diagonal-crossing chunks
                        # (exact zeroing of k > q on the consumed strip)
                        mask_chunk(0)
                        mask_chunk(1)
                    es_tiles[p] = es

                ob = ob_pool.tile([128, QBLK], F32, name=f"ob_{j}", tag="ob")

                def finalize_qs(qs):
                    # divide by the accumulated denominator (col DV). The
                    # reciprocal runs on DVE; the [128,128] scale runs on the
                    # idle Pool engine mid-stream so ACT/DVE stay on the exp
                    # path. The very last sub-block's scale runs on DVE (idle
                    # by then, and faster than Pool) to shorten the tail.
                    rc = rc_pool.tile([128, 1], F32, name=f"rc_{j}_{qs}", tag="rc")
                    rec = nc.vector.reciprocal(rc[:], po[qs][:, DV:DV + 1])
                    if last_tt[0] is not None:
                        bass_rust.add_dep_helper(
                            rec.ins, last_tt[0].ins, sync=False,
                            reason="keep DVE FIFO in completion order",
                        )
                    dst = ob[:, 128 * qs:128 * (qs + 1)]
                    if j == NQB - 1 and qs % 2 == 1:
                        # odd qs on DVE in the last block: two engines drain
                        # the four scales in parallel instead of one serial
                        # Pool queue
                        nc.vector.tensor_scalar_mul(dst, po[qs][:, 0:DV], rc[:])
                    else:
                        nc.gpsimd.tensor_scalar_mul(dst, po[qs][:, 0:DV], rc[:])

                def emit_av_pair(p):
                    es = es_tiles.pop(p)
                    for h in range(2):
                        k = 2 * p + h
                        for qs in range(4):
                            last = 4 * j + qs
                            if k <= last:
                                nc.tensor.matmul(
                                    po[qs][:],
                                    es[:, QBLK * h + 128 * qs:QBLK * h + 128 * (qs + 1)],
                                    vp[:, VW * k:VW * (k + 1)],
                                    start=(k == 0),
                                    stop=(k == last),
                                )
                                if k == last:
                                    finalize_qs(qs)

                for p in range(npr + 4):
                    if p < npr:
                        emit_s_pair(p)
                    if p >= 4:
                        emit_av_pair(p - 4)

                if j == NQB - 1:
                    # split the last store so qs0-2 ship while qs3 finishes
                    nc.sync.dma_start(
                        o_d[:, 512 * j:512 * j + 256], ob[:, 0:256]
                    )
                    nc.scalar.dma_start(
                        o_d[:, 512 * j + 256:512 * (j + 1)], ob[:, 256:QBLK]
                    )
                else:
                    nc.sync.dma_start(
                        o_d[:, 512 * j:512 * (j + 1)], ob[:]
                    )

    nc.compile()
    return nc


def _make_in_maps(Q, K, V):
    ones = np.ones((S, 1), dtype=np.float32)
    # base causal mask tile: BM[k_l, c] = (c >= k_l); shifted views cover all
    # diagonal-crossing chunks
    kk = np.arange(KCH)[:, None]
    qq = np.arange(QBLK)[None, :]
    bm = (qq >= kk).astype(ml_dtypes.bfloat16)
    in_maps = []
    for b in range(Q.shape[0]):
        vp = np.concatenate([V[b], ones], axis=1).astype(ml_dtypes.bfloat16)
        # [S,129] -> [128, 16*129]: partition p holds V rows {128n+p}
        vp_sw = np.ascontiguousarray(
            vp.reshape(NKC, 128, VW).transpose(1, 0, 2).reshape(128, NKC * VW)
        )
        in_maps.append(
            {
                "QT": np.ascontiguousarray(Q[b].T).astype(ml_dtypes.bfloat16),
                "KT": np.ascontiguousarray(K[b].T).astype(ml_dtypes.bfloat16),
                "Vp": vp_sw,
                "BM": bm,
            }
        )
    return in_maps


def _unswizzle_out(o_raw):
    # o_raw [128, 16*128]: O[128*g + p, d] = o_raw[p, 128g + d]
    return np.ascontiguousarray(
        o_raw.reshape(128, NKC, DV).transpose(1, 0, 2).reshape(S, DV)
    )


def _mask_is_causal(mask):
    """True if the mask behaves exactly like the standard causal mask: 0 on
    and below the diagonal, very negative (exp underflows to 0) above."""
    m = np.asarray(mask, dtype=np.float32)
    if m.shape != (1, S, S):
        return False
    m = m[0]
    tril = np.tril_indices(S)
    if not np.all(m[tril] == 0.0):
        return False
    triu = np.triu_indices(S, 1)
    return bool(np.all(m[triu] <= -1e4))


def _host_reference(Q, K, V, mask):
    out = np.empty((Q.shape[0], S, DV), dtype=np.float32)
    for b in range(Q.shape[0]):
        s = (Q[b] @ K[b].T) / math.sqrt(DK) + mask[0]
        s -= s.max(axis=-1, keepdims=True)
        e = np.exp(s)
        out[b] = (e / e.sum(axis=-1, keepdims=True)) @ V[b]
    return out


def kernel(Q, K, V, mask):
    Q = np.asarray(Q, dtype=np.float32)
    K = np.asarray(K, dtype=np.float32)
    V = np.asarray(V, dtype=np.float32)
    mask = np.asarray(mask, dtype=np.float32)

    if not _mask_is_causal(mask):
        # unexpected mask: exact (slow) host path
        return _host_reference(Q, K, V, mask)

    if "nc" not in _CACHE:
        _CACHE["nc"] = _build()
    nc = _CACHE["nc"]

    in_maps = _make_in_maps(Q, K, V)
    res = run_bass_kernel_spmd(nc, in_maps, core_ids=list(range(N_CORES)))
    out = np.stack(
        [_unswizzle_out(res.results[b]["O"]) for b in range(B)]
    ).astype(np.float32)
    return out
